# revision 4
# baseline (speedup 1.0000x reference)
"""AttnDecoderRNN (single GRU step + general attention + output head) on 8 trn2 cores.

Data-parallel over batch B=128 -> 16 per core. All weights replicated.
Host pre-transposes weights and encoder_outputs so every PE matmul has its
contraction dim on partitions.

Math (per core, b = local batch 0..15):
  gi = x @ W_ih.T ; gh = h @ W_hh.T          (PSUM-accumulated matmuls)
  r = sig(gi_r+gh_r+b_r), z = sig(gi_z+gh_z+b_z), n = tanh(i_n+b_in + r*(h_n+b_hn))
  rnn = n + z*(h-n)
  q = rnn @ W_attn                            (b_attn shifts energies per-b only,
                                               softmax-invariant -> dropped)
  e[b,s] = sum_d enc[s,b,d] * q[b,d]          (== reference energies up to const)
  w = softmax(e) ; ctx[b,:] = sum_s w[b,s] enc[s,b,:]
  out = sig(tanh([rnn,ctx] @ W_cat.T + b_cat) @ W_out.T + b_out)
"""

import numpy as np
from contextlib import ExitStack

import concourse.bass as bass
import concourse.tile as tile
from concourse import bacc, mybir
from concourse.bass import ts, ds
from concourse.bass_utils import run_bass_kernel_spmd
from concourse.masks import make_identity

B, S, H = 128, 512, 1024
NCORES = 8
BS = B // NCORES  # 16
F32 = mybir.dt.float32
AF = mybir.ActivationFunctionType
OP = mybir.AluOpType

_cached = {}


def _build_kernel(tc: tile.TileContext, io: dict):
    nc = tc.nc
    with ExitStack() as ctx:
        const = ctx.enter_context(tc.tile_pool(name="const", bufs=1))
        sb = ctx.enter_context(tc.tile_pool(name="sb", bufs=1))
        wpool = ctx.enter_context(tc.tile_pool(name="w", bufs=3))
        epool = ctx.enter_context(tc.tile_pool(name="enc", bufs=3))
        rows = ctx.enter_context(tc.tile_pool(name="rows", bufs=3))

        ident = const.tile([128, 128], F32)
        make_identity(nc, ident[:])

        # ---- small inputs
        xT = sb.tile([128, 8, BS], F32)  # xT[p, c, b] = x[b, c*128+p]
        nc.sync.dma_start(xT[:], io["xT"].rearrange("(c p) b -> p c b", p=128))
        hT = sb.tile([128, 8, BS], F32)
        nc.sync.dma_start(hT[:], io["hT"].rearrange("(c p) b -> p c b", p=128))
        h_nat = sb.tile([BS, H], F32)
        nc.sync.dma_start(h_nat[:], io["h_nat"][:])
        bias_rz = sb.tile([BS, 2 * H], F32)
        nc.sync.dma_start(bias_rz[:], io["bias_rz"][:])
        bias_in = sb.tile([BS, H], F32)
        nc.sync.dma_start(bias_in[:], io["bias_in"][:])
        bias_hn = sb.tile([BS, H], F32)
        nc.sync.dma_start(bias_hn[:], io["bias_hn"][:])
        bias_cat = sb.tile([BS, H], F32)
        nc.sync.dma_start(bias_cat[:], io["bias_cat"][:])
        bias_out = sb.tile([BS, H], F32)
        nc.sync.dma_start(bias_out[:], io["bias_out"][:])

        rnn_pad = sb.tile([128, H], F32)
        nc.vector.memset(rnn_pad[:], 0.0)
        q_pad = sb.tile([128, H], F32)
        nc.vector.memset(q_pad[:], 0.0)
        w_pad = sb.tile([128, S], F32)
        nc.vector.memset(w_pad[:], 0.0)
        ctx_pad = sb.tile([128, H], F32)
        nc.vector.memset(ctx_pad[:], 0.0)
        cat_pad = sb.tile([128, H], F32)
        nc.vector.memset(cat_pad[:], 0.0)

        # ================= Phase A: GRU =================
        with tc.tile_pool(name="psA", bufs=1, space="PSUM") as psA:
            ps_r = psA.tile([BS, H], F32, tag="r")
            ps_z = psA.tile([BS, H], F32, tag="z")
            ps_in = psA.tile([BS, H], F32, tag="in")
            ps_hn = psA.tile([BS, H], F32, tag="hn")
            for c in range(8):
                wih_c = wpool.tile([128, 3 * H], F32, tag="w")
                nc.sync.dma_start(wih_c[:], io["wihT"][ts(c, 128), :])
                whh_c = wpool.tile([128, 3 * H], F32, tag="w")
                nc.sync.dma_start(whh_c[:], io["whhT"][ts(c, 128), :])
                first, last = c == 0, c == 7
                for half in range(2):
                    nsl = ds(half * 512, 512)
                    # r gate: cols 0:1024 ; z gate: cols 1024:2048
                    nc.tensor.matmul(ps_r[:, nsl], xT[:, c, :],
                                     wih_c[:, ds(half * 512, 512)],
                                     start=first, stop=False)
                    nc.tensor.matmul(ps_r[:, nsl], hT[:, c, :],
                                     whh_c[:, ds(half * 512, 512)],
                                     start=False, stop=last)
                    nc.tensor.matmul(ps_z[:, nsl], xT[:, c, :],
                                     wih_c[:, ds(H + half * 512, 512)],
                                     start=first, stop=False)
                    nc.tensor.matmul(ps_z[:, nsl], hT[:, c, :],
                                     whh_c[:, ds(H + half * 512, 512)],
                                     start=False, stop=last)
                    nc.tensor.matmul(ps_in[:, nsl], xT[:, c, :],
                                     wih_c[:, ds(2 * H + half * 512, 512)],
                                     start=first, stop=last)
                    nc.tensor.matmul(ps_hn[:, nsl], hT[:, c, :],
                                     whh_c[:, ds(2 * H + half * 512, 512)],
                                     start=first, stop=last)

            # gates
            r_sb = sb.tile([BS, H], F32, tag="r_sb")
            z_sb = sb.tile([BS, H], F32, tag="z_sb")
            n_sb = sb.tile([BS, H], F32, tag="n_sb")
            t1 = sb.tile([BS, H], F32, tag="t1")
            t2 = sb.tile([BS, H], F32, tag="t2")
            t3 = sb.tile([BS, H], F32, tag="t3")
            nc.vector.tensor_tensor(t1[:], ps_r[:], bias_rz[:, 0:H], OP.add)
            nc.scalar.activation(r_sb[:], t1[:], AF.Sigmoid)
            nc.vector.tensor_tensor(t2[:], ps_z[:], bias_rz[:, H:2 * H], OP.add)
            nc.scalar.activation(z_sb[:], t2[:], AF.Sigmoid)
            nc.vector.tensor_tensor(t3[:], ps_hn[:], bias_hn[:], OP.add)  # h_n
            nc.vector.tensor_tensor(t3[:], r_sb[:], t3[:], OP.mult)      # r*h_n
            nc.vector.tensor_tensor(t1[:], ps_in[:], bias_in[:], OP.add)  # i_n
            nc.vector.tensor_tensor(t1[:], t1[:], t3[:], OP.add)
            nc.scalar.activation(n_sb[:], t1[:], AF.Tanh)
            nc.vector.tensor_tensor(t2[:], h_nat[:], n_sb[:], OP.subtract)
            nc.vector.tensor_tensor(t2[:], z_sb[:], t2[:], OP.mult)
            nc.vector.tensor_tensor(rnn_pad[0:BS, :], n_sb[:], t2[:], OP.add)

        nc.sync.dma_start(io["hid"][:], rnn_pad[0:BS, :])

        # ================= Phase B1: rnn^T, q, q^T =================
        rnnT = sb.tile([128, 8, BS], F32)
        qT = sb.tile([128, 8, BS], F32)
        with tc.tile_pool(name="psB1", bufs=1, space="PSUM") as psB1:
            for c in range(8):
                tp = psB1.tile([128, 128], F32, tag="tpa" if c % 2 == 0 else "tpb")
                nc.tensor.transpose(tp[:], rnn_pad[:, ts(c, 128)], ident[:])
                nc.vector.tensor_copy(rnnT[:, c, :], tp[:, 0:BS])
            ps_q = psB1.tile([BS, H], F32, tag="q")
            for c in range(8):
                wa_c = wpool.tile([128, H], F32, tag="w1")
                nc.sync.dma_start(wa_c[:], io["wattn"][ts(c, 128), :])
                for half in range(2):
                    nc.tensor.matmul(ps_q[:, ds(half * 512, 512)], rnnT[:, c, :],
                                     wa_c[:, ds(half * 512, 512)],
                                     start=(c == 0), stop=(c == 7))
            nc.vector.tensor_copy(q_pad[0:BS, :], ps_q[:])
            for c in range(8):
                tp = psB1.tile([128, 128], F32, tag="tpa" if c % 2 == 0 else "tpb")
                nc.tensor.transpose(tp[:], q_pad[:, ts(c, 128)], ident[:])
                nc.vector.tensor_copy(qT[:, c, :], tp[:, 0:BS])

        # ================= Phase B2: energies + softmax + context =================
        e_sb = sb.tile([BS, S], F32)
        wT = sb.tile([128, 4, BS], F32)
        with tc.tile_pool(name="psB2", bufs=1, space="PSUM") as psB2:
            for b in range(BS):
                et = epool.tile([128, 8, 512], F32, tag="enc")
                nc.sync.dma_start(
                    et[:], io["enc_t"][b].rearrange("(c p) s -> p c s", p=128))
                ps_e = psB2.tile([1, S], F32, tag="ea" if b % 2 == 0 else "eb")
                for c in range(8):
                    nc.tensor.matmul(ps_e[:], qT[:, c, b:b + 1], et[:, c, :],
                                     start=(c == 0), stop=(c == 7))
                e_row = rows.tile([1, S], F32, tag="e_row")
                nc.vector.tensor_copy(e_row[:], ps_e[:])
                nc.sync.dma_start(e_sb[b:b + 1, :], e_row[:])

            # softmax over s (per-partition row b)
            negmax = sb.tile([BS, 1], F32)
            nc.vector.tensor_reduce(out=negmax[:], in_=e_sb[:],
                                    op=OP.max, axis=mybir.AxisListType.X,
                                    negate=True)
            denom = sb.tile([BS, 1], F32)
            nc.scalar.activation(w_pad[0:BS, :], e_sb[:], AF.Exp,
                                 bias=negmax[:], accum_out=denom[:])
            rec = sb.tile([BS, 1], F32)
            nc.vector.reciprocal(rec[:], denom[:])
            nc.vector.tensor_scalar_mul(w_pad[0:BS, :], w_pad[0:BS, :], rec[:])
            nc.sync.dma_start(io["attn"][:], w_pad[0:BS, :])

            for c in range(4):
                tp = psB2.tile([128, 128], F32, tag="tpa" if c % 2 == 0 else "tpb")
                nc.tensor.transpose(tp[:], w_pad[:, ts(c, 128)], ident[:])
                nc.vector.tensor_copy(wT[:, c, :], tp[:, 0:BS])

            for b in range(BS):
                en = epool.tile([128, 4, 1024], F32, tag="enc")
                nc.sync.dma_start(
                    en[:], io["enc_n"][b].rearrange("(c p) d -> p c d", p=128))
                ps_c = psB2.tile([1, H], F32, tag="ca" if b % 2 == 0 else "cb")
                for c in range(4):
                    for half in range(2):
                        nc.tensor.matmul(ps_c[:, ds(half * 512, 512)],
                                         wT[:, c, b:b + 1],
                                         en[:, c, ds(half * 512, 512)],
                                         start=(c == 0), stop=(c == 3))
                c_row = rows.tile([1, H], F32, tag="c_row")
                nc.scalar.copy(c_row[:], ps_c[:])
                nc.sync.dma_start(ctx_pad[b:b + 1, :], c_row[:])

        # ================= Phase C: output head =================
        with tc.tile_pool(name="psC", bufs=1, space="PSUM") as psC:
            ctxT = sb.tile([128, 8, BS], F32)
            for c in range(8):
                tp = psC.tile([128, 128], F32, tag="tpa" if c % 2 == 0 else "tpb")
                nc.tensor.transpose(tp[:], ctx_pad[:, ts(c, 128)], ident[:])
                nc.vector.tensor_copy(ctxT[:, c, :], tp[:, 0:BS])

            ps_cat = psC.tile([BS, H], F32, tag="cat")
            for g in range(16):
                wc_g = wpool.tile([128, H], F32, tag="w1")
                nc.sync.dma_start(wc_g[:], io["wcatT"][ts(g, 128), :])
                lhsT = rnnT[:, g, :] if g < 8 else ctxT[:, g - 8, :]
                for half in range(2):
                    nc.tensor.matmul(ps_cat[:, ds(half * 512, 512)], lhsT,
                                     wc_g[:, ds(half * 512, 512)],
                                     start=(g == 0), stop=(g == 15))
            t4 = sb.tile([BS, H], F32, tag="t4")
            nc.vector.tensor_tensor(t4[:], ps_cat[:], bias_cat[:], OP.add)
            nc.scalar.activation(cat_pad[0:BS, :], t4[:], AF.Tanh)

            catT = sb.tile([128, 8, BS], F32)
            for c in range(8):
                tp = psC.tile([128, 128], F32, tag="tpa" if c % 2 == 0 else "tpb")
                nc.tensor.transpose(tp[:], cat_pad[:, ts(c, 128)], ident[:])
                nc.vector.tensor_copy(catT[:, c, :], tp[:, 0:BS])

            ps_out = psC.tile([BS, H], F32, tag="out")
            for c in range(8):
                wo_c = wpool.tile([128, H], F32, tag="w1")
                nc.sync.dma_start(wo_c[:], io["woutT"][ts(c, 128), :])
                for half in range(2):
                    nc.tensor.matmul(ps_out[:, ds(half * 512, 512)], catT[:, c, :],
                                     wo_c[:, ds(half * 512, 512)],
                                     start=(c == 0), stop=(c == 7))
            t5 = sb.tile([BS, H], F32, tag="t5")
            nc.vector.tensor_tensor(t5[:], ps_out[:], bias_out[:], OP.add)
            out_sb = sb.tile([BS, H], F32, tag="out_sb")
            nc.scalar.activation(out_sb[:], t5[:], AF.Sigmoid)
            nc.sync.dma_start(io["out"][:], out_sb[:])


def build_nc():
    if "nc" in _cached:
        return _cached["nc"]
    nc = bacc.Bacc("TRN2", target_bir_lowering=False, debug=False,
                   num_devices=NCORES)
    io = {}
    in_specs = [
        ("xT", [H, BS]), ("hT", [H, BS]), ("h_nat", [BS, H]),
        ("enc_t", [BS, H, S]), ("enc_n", [BS, S, H]),
        ("wihT", [H, 3 * H]), ("whhT", [H, 3 * H]), ("wattn", [H, H]),
        ("wcatT", [2 * H, H]), ("woutT", [H, H]),
        ("bias_rz", [BS, 2 * H]), ("bias_in", [BS, H]), ("bias_hn", [BS, H]),
        ("bias_cat", [BS, H]), ("bias_out", [BS, H]),
    ]
    for name, shape in in_specs:
        io[name] = nc.dram_tensor(name, shape, F32, kind="ExternalInput").ap()
    for name, shape in [("out", [BS, H]), ("hid", [BS, H]), ("attn", [BS, S])]:
        io[name] = nc.dram_tensor(name, shape, F32, kind="ExternalOutput").ap()
    with tile.TileContext(nc) as tc:
        _build_kernel(tc, io)
    nc.compile()
    _cached["nc"] = nc
    return nc


def make_in_maps(input_seq, last_hidden, encoder_outputs,
                 W_ih, b_ih, W_hh, b_hh, W_attn, b_attn,
                 W_concat, b_concat, W_out, b_out):
    f = np.float32
    wihT = np.ascontiguousarray(W_ih.T, dtype=f)
    whhT = np.ascontiguousarray(W_hh.T, dtype=f)
    wattn = np.ascontiguousarray(W_attn, dtype=f)
    wcatT = np.ascontiguousarray(W_concat.T, dtype=f)
    woutT = np.ascontiguousarray(W_out.T, dtype=f)
    b3 = (np.asarray(b_ih) + np.asarray(b_hh)).astype(f)
    bias_rz = np.tile(b3[None, :2 * H], (BS, 1))
    bias_in = np.tile(np.asarray(b_ih, dtype=f)[None, 2 * H:], (BS, 1))
    bias_hn = np.tile(np.asarray(b_hh, dtype=f)[None, 2 * H:], (BS, 1))
    bias_cat = np.tile(np.asarray(b_concat, dtype=f)[None, :], (BS, 1))
    bias_out = np.tile(np.asarray(b_out, dtype=f)[None, :], (BS, 1))
    x = np.asarray(input_seq, dtype=f)          # [B, H]
    h = np.asarray(last_hidden, dtype=f)[0]     # [B, H]
    enc = np.asarray(encoder_outputs, dtype=f)  # [S, B, H]
    enc_bsh = enc.transpose(1, 0, 2)            # [B, S, H]
    enc_bhs = enc.transpose(1, 2, 0)            # [B, H, S]
    in_maps = []
    for core in range(NCORES):
        bsl = slice(core * BS, (core + 1) * BS)
        in_maps.append({
            "xT": np.ascontiguousarray(x[bsl].T),
            "hT": np.ascontiguousarray(h[bsl].T),
            "h_nat": np.ascontiguousarray(h[bsl]),
            "enc_t": np.ascontiguousarray(enc_bhs[bsl]),
            "enc_n": np.ascontiguousarray(enc_bsh[bsl]),
            "wihT": wihT, "whhT": whhT, "wattn": wattn,
            "wcatT": wcatT, "woutT": woutT,
            "bias_rz": bias_rz, "bias_in": bias_in, "bias_hn": bias_hn,
            "bias_cat": bias_cat, "bias_out": bias_out,
        })
    return in_maps


def assemble_outputs(results):
    output = np.concatenate([r["out"] for r in results], axis=0)
    hidden = np.concatenate([r["hid"] for r in results], axis=0)[None]
    attn = np.concatenate([r["attn"] for r in results], axis=0)[:, None, :]
    return output, hidden, attn


def kernel(**inputs):
    nc = build_nc()
    in_maps = make_in_maps(**inputs)
    res = run_bass_kernel_spmd(nc, in_maps, core_ids=list(range(NCORES)))
    return assemble_outputs(res.results)
